# revision 2
# baseline (speedup 1.0000x reference)
"""CLIP cross-attention kernel for 8 TRN2 NeuronCores (v2).

Math (per batch b, head h):
  Q = (T @ Wq + bq) * scale           T = text_states[:, b, :]   (128, 1024)
  K = X @ Wk + bk                     X = hidden_states[b]       (4096, 1024)
  V = X @ Wv + bv
  S = Q_h @ K_h^T                     (128, 4096)
  E = exp(S); d = rowsum(E)
  out_h = E^T @ (E @ V_h) / d^2       (4096, 64)
  final = concat_h(out_h) @ Wo + bo

Sharding: batch across 8 cores (2 batches each), weights replicated.

v2 design vs baseline:
 - K^T and V stay RESIDENT in SBUF (no DRAM round trip, no descriptor storm).
   X is streamed in 512-column chunks during the K/V projections.
 - K^T and Q^T are stored as fp8e4 (scaled by 8): S matmuls run fp8,
   kt shrinks to 32KB/partition. Verified numerically: rel err ~7.8e-3.
 - E^T is produced by the DMA xbar transpose engine (dma_start_transpose)
   from E, replacing the S^T matmul pass + second exp: saves ~135us of PE
   and ~170us of ACT per core.
 - Attention head loop is software-pipelined (emit S(h); consume(h-1))
   so the PE never waits on the ACT exp / DVE normalization chain.
 - All pools hoisted to top level; phases of consecutive batches overlap.
"""
import sys
import numpy as np

sys.path.insert(0, '/opt/trn_rl_repo')

import concourse.bass as bass          # noqa: E402
import concourse.tile as tile          # noqa: E402
from concourse import bacc, mybir      # noqa: E402
from concourse import bass_utils       # noqa: E402
from contextlib import ExitStack       # noqa: E402

DT = mybir.dt.float32
BF = mybir.dt.bfloat16
F8 = mybir.dt.float8e4
AF = mybir.ActivationFunctionType

B, LT, LV, D, H = 16, 128, 4096, 1024, 16
HD = D // H          # 64
NB = 2               # batches per core
N_CORES = 8
SCALE = HD ** -0.5
KD = D // 128        # 8
LVT = LV // 128      # 32
NCH = LV // 512      # 8
QK8 = 8.0            # fp8 storage scale for q/k


def build_program(nb=NB, with_bv=False):
    nc = bacc.Bacc("TRN2", target_bir_lowering=False, debug=False)

    xt_d = nc.dram_tensor("xt", [nb, D, LV], BF, kind="ExternalInput")
    tt_d = nc.dram_tensor("tt", [nb, D, LT], BF, kind="ExternalInput")
    w_d = {nm: nc.dram_tensor(nm, [D, D], BF, kind="ExternalInput")
           for nm in ("wq", "wk", "wv", "wo")}
    b_d = {nm: nc.dram_tensor(nm, [D], DT, kind="ExternalInput")
           for nm in ("bqs8", "bk8", "bo")}
    if with_bv:
        b_d["bv"] = nc.dram_tensor("bv", [D], DT, kind="ExternalInput")
    out_d = nc.dram_tensor("out", [nb, D, LV], DT, kind="ExternalOutput")
    ot_d = nc.dram_tensor("ot_scratch", [nb, D, LV], BF)

    with tile.TileContext(nc) as tc, ExitStack() as top:
        ep = top.enter_context
        wpool = ep(tc.tile_pool(name="wp", bufs=2))
        biasp = ep(tc.tile_pool(name="biasp", bufs=1))
        xtp = ep(tc.tile_pool(name="xtp", bufs=2))
        ktp = ep(tc.tile_pool(name="ktp", bufs=1))
        vp = ep(tc.tile_pool(name="vp", bufs=1))
        ttp = ep(tc.tile_pool(name="ttp", bufs=1))
        qtp = ep(tc.tile_pool(name="qtp", bufs=1))
        enp = ep(tc.tile_pool(name="enp", bufs=2))
        etp = ep(tc.tile_pool(name="etp", bufs=2))
        smp = ep(tc.tile_pool(name="smp", bufs=2))
        otsg = ep(tc.tile_pool(name="otsg", bufs=3))
        fosg = ep(tc.tile_pool(name="fosg", bufs=3))
        otip = ep(tc.tile_pool(name="otip", bufs=2))
        psA = ep(tc.tile_pool(name="psA", bufs=2, space="PSUM"))
        psB = ep(tc.tile_pool(name="psB", bufs=2, space="PSUM"))
        psC = ep(tc.tile_pool(name="psC", bufs=2, space="PSUM"))

        def load_weight(nm):
            t = wpool.tile([128, KD, D], BF, name=f"w_{nm}", tag="w")
            src = w_d[nm].ap().rearrange("(k p) n -> p k n", p=128)
            for k in range(KD):
                nc.sync.dma_start(t[:, k, :], src[:, k, :])
            return t

        bias_sb = {}
        for nm in b_d:
            t = biasp.tile([128, KD], DT, name=f"b_{nm}", tag=f"b_{nm}")
            nc.sync.dma_start(t[:], b_d[nm].ap().rearrange("(k p) -> p k", p=128))
            bias_sb[nm] = t

        bv_bcast = None
        if with_bv:
            bv_row = biasp.tile([1, D], DT, tag="bv_row")
            nc.sync.dma_start(bv_row[:], b_d["bv"].ap().unsqueeze(0))
            ones_row = biasp.tile([1, 128], DT, tag="ones_row")
            nc.vector.memset(ones_row[:], 1.0)
            bv_bcast = biasp.tile([128, D], DT, tag="bv_bcast")
            for g in range(2):
                pb = psB.tile([128, 512], DT, name="bv_ps", tag="B")
                nc.tensor.matmul(pb[:], ones_row[:],
                                 bv_row[:, 512 * g:512 * (g + 1)])
                nc.vector.tensor_copy(bv_bcast[:, 512 * g:512 * (g + 1)], pb[:])

        for b in range(nb):
            # ---------- P1: K^T (fp8, resident) + V (bf16, resident) ----------
            wk_sb = load_weight("wk")
            wv_sb = load_weight("wv")
            kt_sb = ktp.tile([128, KD, LV], F8, name="kt", tag="kt")
            v_sb = vp.tile([128, LVT, D], BF, name="v", tag="v")
            xsrc = xt_d[b].rearrange("(k p) n -> p k n", p=128)

            for cp in range(LV // 1024):
                xts = []
                for half in range(2):
                    c0 = 1024 * cp + 512 * half
                    xt_t = xtp.tile([128, KD, 512], BF, name="xt_c", tag="xt")
                    nc.sync.dma_start(xt_t[:], xsrc[:, :, c0:c0 + 512])
                    xts.append(xt_t)
                # K^T: out rows m-block, cols = this 1024-chunk
                for m in range(KD):
                    ps = psA.tile([128, 1024], DT, name="k_ps", tag="A")
                    for k in range(KD):
                        lw = wk_sb[:, k, 128 * m:128 * (m + 1)]
                        for half in range(2):
                            nc.tensor.matmul(ps[:, 512 * half:512 * (half + 1)],
                                             lw, xts[half][:, k, :],
                                             start=(k == 0), stop=(k == KD - 1))
                    nc.scalar.activation(
                        kt_sb[:, m, 1024 * cp:1024 * (cp + 1)], ps[:],
                        AF.Identity, bias=bias_sb["bk8"][:, m:m + 1], scale=QK8)
                # V: vis-tiles of this chunk pair, natural layout
                for half in range(2):
                    for tl in range(4):
                        t_abs = 8 * cp + 4 * half + tl
                        for g in range(2):
                            ps = psB.tile([128, 512], DT, name="v_ps", tag="B")
                            for k in range(KD):
                                nc.tensor.matmul(
                                    ps[:],
                                    xts[half][:, k, 128 * tl:128 * (tl + 1)],
                                    wv_sb[:, k, 512 * g:512 * (g + 1)],
                                    start=(k == 0), stop=(k == KD - 1))
                            dst = v_sb[:, t_abs, 512 * g:512 * (g + 1)]
                            if with_bv:
                                nc.vector.tensor_add(
                                    dst, ps[:], bv_bcast[:, 512 * g:512 * (g + 1)])
                            else:
                                nc.vector.tensor_copy(dst, ps[:])

            # ---------- P2: Q^T (fp8) ----------
            tt_sb = ttp.tile([128, KD, LT], BF, name="tt", tag="tt")
            nc.sync.dma_start(tt_sb[:], tt_d[b].rearrange("(k p) t -> p k t", p=128))
            wq_sb = load_weight("wq")
            qt_sb = qtp.tile([128, KD, LT], F8, name="qt", tag="qt")
            for m in range(KD):
                ps = psC.tile([128, LT], DT, name="q_ps", tag="C")
                for k in range(KD):
                    nc.tensor.matmul(ps[:], wq_sb[:, k, 128 * m:128 * (m + 1)],
                                     tt_sb[:, k, :],
                                     start=(k == 0), stop=(k == KD - 1))
                nc.scalar.activation(qt_sb[:, m, :], ps[:], AF.Identity,
                                     bias=bias_sb["bqs8"][:, m:m + 1],
                                     scale=SCALE * QK8)

            wo_sb = load_weight("wo")   # prefetch during attention

            # ---------- P3: attention, software-pipelined over heads ----------
            live = {}

            def emit_produce(h):
                p, hb = h // 2, 64 * (h % 2)
                qth = qt_sb[hb:hb + 64, p, :]
                en = enp.tile([128, LV], BF, name="en", tag="en")
                dparts = smp.tile([128, 4], DT, name="dparts", tag="dparts")
                for g in range(4):
                    ps = psA.tile([128, 1024], DT, name="s_ps", tag="A")
                    for half in range(2):
                        c0 = 1024 * g + 512 * half
                        nc.tensor.matmul(ps[:, 512 * half:512 * (half + 1)],
                                         qth, kt_sb[hb:hb + 64, p, c0:c0 + 512])
                    nc.scalar.activation(
                        en[:, 1024 * g:1024 * (g + 1)], ps[:], AF.Exp,
                        scale=1.0 / (QK8 * QK8),
                        accum_out=dparts[:, g:g + 1])
                et = etp.tile([128, LVT, 128], BF, name="et", tag="et")
                for g in range(4):
                    nc.sync.dma_start_transpose(
                        et[:, 8 * g:8 * (g + 1), :],
                        en[:, 1024 * g:1024 * (g + 1)])
                live[h] = (en, et, dparts)

            def emit_consume(h):
                en, et, dparts = live.pop(h)
                ub = psC.tile([128, HD], DT, name="u_ps", tag="C")
                for t in range(LVT):
                    nc.tensor.matmul(ub[:], et[:, t, :],
                                     v_sb[:, t, HD * h:HD * (h + 1)],
                                     start=(t == 0), stop=(t == LVT - 1))
                dsum = smp.tile([128, 1], DT, name="dsum", tag="dsum")
                nc.vector.reduce_sum(dsum[:], dparts[:], axis=mybir.AxisListType.X)
                rd = smp.tile([128, 1], DT, name="rd", tag="rd")
                nc.vector.reciprocal(rd[:], dsum[:])
                rr = smp.tile([128, 1], DT, name="rr", tag="rr")
                nc.vector.tensor_mul(rr[:], rd[:], rd[:])
                up = smp.tile([128, HD], BF, name="up", tag="up")
                nc.vector.tensor_scalar_mul(up[:], ub[:], rr[:])
                for n in range(NCH):
                    ob = psB.tile([64, 512], DT, name="ot_ps", tag="B")
                    nc.tensor.matmul(ob[:], up[:], en[:, 512 * n:512 * (n + 1)])
                    ost = otsg.tile([64, 512], BF, name="ot_st", tag="ot_st")
                    nc.vector.tensor_copy(ost[:], ob[:])
                    nc.sync.dma_start(
                        ot_d[b, 64 * h:64 * (h + 1), 512 * n:512 * (n + 1)],
                        ost[:])

            for h in range(H):
                emit_produce(h)
                if h > 0:
                    emit_consume(h - 1)
            emit_consume(H - 1)

            # ---------- P4: final projection ----------
            osrc = ot_d[b].rearrange("(k p) n -> p k n", p=128)
            for c in range(NCH):
                oti = otip.tile([128, KD, 512], BF, name="oti", tag="oti")
                nc.sync.dma_start(oti[:], osrc[:, :, 512 * c:512 * (c + 1)])
                for m in range(KD):
                    ps = psA.tile([128, 512], DT, name="f_ps", tag="A")
                    for k in range(KD):
                        nc.tensor.matmul(ps[:], wo_sb[:, k, 128 * m:128 * (m + 1)],
                                         oti[:, k, :],
                                         start=(k == 0), stop=(k == KD - 1))
                    st = fosg.tile([128, 512], DT, name="fin_st", tag="fin_st")
                    nc.scalar.activation(st[:], ps[:], AF.Identity,
                                         bias=bias_sb["bo"][:, m:m + 1])
                    nc.sync.dma_start(
                        out_d[b, 128 * m:128 * (m + 1), 512 * c:512 * (c + 1)],
                        st[:])

    nc.compile()
    return nc


_nc_cache = {}


def _get_program(nb=NB, with_bv=False):
    key = (nb, with_bv)
    if key not in _nc_cache:
        _nc_cache[key] = build_program(nb, with_bv)
    return _nc_cache[key]


def make_in_maps(hidden_states, text_states, Wq, bq, Wk, bk, Wv, bv, Wo, bo):
    """Host-side staging: transpose to feature-major, shard batches."""
    import ml_dtypes
    f32 = np.float32
    bf16 = ml_dtypes.bfloat16
    hs = np.asarray(hidden_states, f32)
    ts = np.asarray(text_states, f32)
    xt_all = np.ascontiguousarray(hs.transpose(0, 2, 1)).astype(bf16)  # (B,D,LV)
    # Faithful to the reference's torch-style .view: text_states (LT, B, D)
    # reinterpreted in raw memory order as (B, LT, D), then feature-major.
    tt_all = np.ascontiguousarray(
        ts.reshape(B, LT, D).transpose(0, 2, 1)).astype(bf16)
    with_bv = bool(np.any(np.asarray(bv)))
    shared = {
        "wq": np.asarray(Wq, f32).astype(bf16),
        "wk": np.asarray(Wk, f32).astype(bf16),
        "wv": np.asarray(Wv, f32).astype(bf16),
        "wo": np.asarray(Wo, f32).astype(bf16),
        "bqs8": np.ascontiguousarray(np.asarray(bq, f32) * (SCALE * QK8)),
        "bk8": np.ascontiguousarray(np.asarray(bk, f32) * QK8),
        "bo": np.ascontiguousarray(np.asarray(bo, f32)),
    }
    if with_bv:
        shared["bv"] = np.ascontiguousarray(np.asarray(bv, f32))
    in_maps = []
    for c in range(N_CORES):
        sl = slice(c * NB, (c + 1) * NB)
        in_maps.append({
            "xt": np.ascontiguousarray(xt_all[sl]),
            "tt": np.ascontiguousarray(tt_all[sl]),
            **shared,
        })
    return in_maps, with_bv


def kernel(hidden_states, text_states, Wq, bq, Wk, bk, Wv, bv, Wo, bo):
    in_maps, with_bv = make_in_maps(hidden_states, text_states, Wq, bq,
                                    Wk, bk, Wv, bv, Wo, bo)
    nc = _get_program(with_bv=with_bv)
    res = bass_utils.run_bass_kernel_spmd(nc, in_maps, list(range(N_CORES)))
    out = np.empty((B, LV, D), np.float32)
    for c in range(N_CORES):
        o = res.results[c]["out"]                                  # (NB, D, LV)
        for j in range(NB):
            out[c * NB + j] = o[j].T
    return out


# revision 4
# speedup vs baseline: 1.0159x; 1.0159x over previous
"""CLIP cross-attention kernel for 8 TRN2 NeuronCores (v2).

Math (per batch b, head h):
  Q = (T @ Wq + bq) * scale           T = text_states[:, b, :]   (128, 1024)
  K = X @ Wk + bk                     X = hidden_states[b]       (4096, 1024)
  V = X @ Wv + bv
  S = Q_h @ K_h^T                     (128, 4096)
  E = exp(S); d = rowsum(E)
  out_h = E^T @ (E @ V_h) / d^2       (4096, 64)
  final = concat_h(out_h) @ Wo + bo

Sharding: batch across 8 cores (2 batches each), weights replicated.

v2 design vs baseline:
 - K^T and V stay RESIDENT in SBUF (no DRAM round trip, no descriptor storm).
   X is streamed in 512-column chunks during the K/V projections.
 - K^T and Q^T are stored as fp8e4 (scaled by 8): S matmuls run fp8,
   kt shrinks to 32KB/partition. Verified numerically: rel err ~7.8e-3.
 - E^T is produced by the DMA xbar transpose engine (dma_start_transpose)
   from E, replacing the S^T matmul pass + second exp: saves ~135us of PE
   and ~170us of ACT per core.
 - Attention head loop is software-pipelined (emit S(h); consume(h-1))
   so the PE never waits on the ACT exp / DVE normalization chain.
 - All pools hoisted to top level; phases of consecutive batches overlap.
"""
import sys
import numpy as np

sys.path.insert(0, '/opt/trn_rl_repo')

import concourse.bass as bass          # noqa: E402
import concourse.tile as tile          # noqa: E402
from concourse import bacc, mybir      # noqa: E402
from concourse import bass_utils       # noqa: E402
from contextlib import ExitStack       # noqa: E402

DT = mybir.dt.float32
BF = mybir.dt.bfloat16
F8 = mybir.dt.float8e4
AF = mybir.ActivationFunctionType

B, LT, LV, D, H = 16, 128, 4096, 1024, 16
HD = D // H          # 64
NB = 2               # batches per core
N_CORES = 8
SCALE = HD ** -0.5
KD = D // 128        # 8
LVT = LV // 128      # 32
NCH = LV // 512      # 8
QK8 = 8.0            # fp8 storage scale for q/k


def build_program(nb=NB, with_bv=False):
    nc = bacc.Bacc("TRN2", target_bir_lowering=False, debug=False)

    xt_d = nc.dram_tensor("xt", [nb, D, LV], BF, kind="ExternalInput")
    tt_d = nc.dram_tensor("tt", [nb, D, LT], BF, kind="ExternalInput")
    w_d = {nm: nc.dram_tensor(nm, [D, D], BF, kind="ExternalInput")
           for nm in ("wq", "wk", "wv", "wo")}
    b_d = {nm: nc.dram_tensor(nm, [D], DT, kind="ExternalInput")
           for nm in ("bqs8", "bk8", "bo")}
    if with_bv:
        b_d["bv"] = nc.dram_tensor("bv", [D], DT, kind="ExternalInput")
    out_d = nc.dram_tensor("out", [nb, D, LV], DT, kind="ExternalOutput")
    ot_d = nc.dram_tensor("ot_scratch", [nb, D, LV], BF)

    with tile.TileContext(nc) as tc, ExitStack() as top:
        ep = top.enter_context
        wpool = ep(tc.tile_pool(name="wp", bufs=2))
        biasp = ep(tc.tile_pool(name="biasp", bufs=1))
        xtp = ep(tc.tile_pool(name="xtp", bufs=2))
        ktp = ep(tc.tile_pool(name="ktp", bufs=1))
        vp = ep(tc.tile_pool(name="vp", bufs=1))
        ttp = ep(tc.tile_pool(name="ttp", bufs=1))
        qtp = ep(tc.tile_pool(name="qtp", bufs=1))
        enp = ep(tc.tile_pool(name="enp", bufs=2))
        etp = ep(tc.tile_pool(name="etp", bufs=2))
        smp = ep(tc.tile_pool(name="smp", bufs=2))
        otsg = ep(tc.tile_pool(name="otsg", bufs=3))
        fosg = ep(tc.tile_pool(name="fosg", bufs=3))
        otip = ep(tc.tile_pool(name="otip", bufs=2))
        psA = ep(tc.tile_pool(name="psA", bufs=2, space="PSUM"))
        psB = ep(tc.tile_pool(name="psB", bufs=2, space="PSUM"))
        psC = ep(tc.tile_pool(name="psC", bufs=2, space="PSUM"))

        def load_weight(nm):
            t = wpool.tile([128, KD, D], BF, name=f"w_{nm}", tag="w")
            src = w_d[nm].ap().rearrange("(k p) n -> p k n", p=128)
            for k in range(KD):
                nc.sync.dma_start(t[:, k, :], src[:, k, :])
            return t

        bias_sb = {}
        for nm in b_d:
            t = biasp.tile([128, KD], DT, name=f"b_{nm}", tag=f"b_{nm}")
            nc.sync.dma_start(t[:], b_d[nm].ap().rearrange("(k p) -> p k", p=128))
            bias_sb[nm] = t

        bv_bcast = None
        if with_bv:
            bv_row = biasp.tile([1, D], DT, tag="bv_row")
            nc.sync.dma_start(bv_row[:], b_d["bv"].ap().unsqueeze(0))
            ones_row = biasp.tile([1, 128], DT, tag="ones_row")
            nc.vector.memset(ones_row[:], 1.0)
            bv_bcast = biasp.tile([128, D], DT, tag="bv_bcast")
            for g in range(2):
                pb = psB.tile([128, 512], DT, name="bv_ps", tag="B")
                nc.tensor.matmul(pb[:], ones_row[:],
                                 bv_row[:, 512 * g:512 * (g + 1)])
                nc.vector.tensor_copy(bv_bcast[:, 512 * g:512 * (g + 1)], pb[:])

        for b in range(nb):
            # ---------- P1: K^T (fp8, resident) + V (bf16, resident) ----------
            p1_scope = nc.named_scope(f"p1_b{b}"); p1_scope.__enter__()
            wk_sb = load_weight("wk")
            wv_sb = load_weight("wv")
            kt_sb = ktp.tile([128, KD, LV], F8, name="kt", tag="kt")
            v_sb = vp.tile([128, LVT, D], BF, name="v", tag="v")
            xsrc = xt_d[b].rearrange("(k p) n -> p k n", p=128)

            for cp in range(LV // 1024):
                xts = []
                for half in range(2):
                    c0 = 1024 * cp + 512 * half
                    xt_t = xtp.tile([128, KD, 512], BF, name="xt_c", tag="xt")
                    nc.sync.dma_start(xt_t[:], xsrc[:, :, c0:c0 + 512])
                    xts.append(xt_t)
                # K^T: out rows m-block, cols = this 1024-chunk
                for m in range(KD):
                    ps = psA.tile([128, 1024], DT, name="k_ps", tag="A")
                    for k in range(KD):
                        lw = wk_sb[:, k, 128 * m:128 * (m + 1)]
                        for half in range(2):
                            nc.tensor.matmul(ps[:, 512 * half:512 * (half + 1)],
                                             lw, xts[half][:, k, :],
                                             start=(k == 0), stop=(k == KD - 1))
                    nc.scalar.activation(
                        kt_sb[:, m, 1024 * cp:1024 * (cp + 1)], ps[:],
                        AF.Identity, bias=bias_sb["bk8"][:, m:m + 1], scale=QK8)
                # V: vis-tiles of this chunk pair, natural layout
                for half in range(2):
                    for tl in range(4):
                        t_abs = 8 * cp + 4 * half + tl
                        for g in range(2):
                            ps = psB.tile([128, 512], DT, name="v_ps", tag="B")
                            for k in range(KD):
                                nc.tensor.matmul(
                                    ps[:],
                                    xts[half][:, k, 128 * tl:128 * (tl + 1)],
                                    wv_sb[:, k, 512 * g:512 * (g + 1)],
                                    start=(k == 0), stop=(k == KD - 1))
                            dst = v_sb[:, t_abs, 512 * g:512 * (g + 1)]
                            if with_bv:
                                nc.vector.tensor_add(
                                    dst, ps[:], bv_bcast[:, 512 * g:512 * (g + 1)])
                            else:
                                nc.vector.tensor_copy(dst, ps[:])

            p1_scope.__exit__(None, None, None)
            # ---------- P2: Q^T (fp8) ----------
            p2_scope = nc.named_scope(f"p2_b{b}"); p2_scope.__enter__()
            tt_sb = ttp.tile([128, KD, LT], BF, name="tt", tag="tt")
            nc.sync.dma_start(tt_sb[:], tt_d[b].rearrange("(k p) t -> p k t", p=128))
            wq_sb = load_weight("wq")
            qt_sb = qtp.tile([128, KD, LT], F8, name="qt", tag="qt")
            for m in range(KD):
                ps = psC.tile([128, LT], DT, name="q_ps", tag="C")
                for k in range(KD):
                    nc.tensor.matmul(ps[:], wq_sb[:, k, 128 * m:128 * (m + 1)],
                                     tt_sb[:, k, :],
                                     start=(k == 0), stop=(k == KD - 1))
                nc.scalar.activation(qt_sb[:, m, :], ps[:], AF.Identity,
                                     bias=bias_sb["bqs8"][:, m:m + 1],
                                     scale=SCALE * QK8)

            wo_sb = load_weight("wo")   # prefetch during attention
            p2_scope.__exit__(None, None, None)
            p3_scope = nc.named_scope(f"p3_b{b}"); p3_scope.__enter__()

            # ---------- P3: attention, software-pipelined over heads ----------
            # Emission order per head h keeps the PE fed while the ACT exp /
            # DVE normalization chains for neighbouring heads complete:
            #   u(h-1)[0:16] | S(h)[g0,g1] | u(h-1)[16:32] | S(h)[g2,g3] |
            #   chain(h-1) | out(h-1) | casts+DMA(h-1)
            live = {}

            def emit_S(h, glo, ghi):
                p, hb = h // 2, 64 * (h % 2)
                qth = qt_sb[hb:hb + 64, p, :]
                if glo == 0:
                    live[h] = {
                        "en": enp.tile([128, LV], BF, name="en", tag="en"),
                        "et": etp.tile([128, LVT, 128], BF, name="et", tag="et"),
                        "dparts": smp.tile([128, 4], DT, name="dparts",
                                           tag="dparts"),
                    }
                st = live[h]
                for g in range(glo, ghi):
                    ps = psA.tile([128, 1024], DT, name="s_ps", tag="A")
                    for half in range(2):
                        c0 = 1024 * g + 512 * half
                        nc.tensor.matmul(ps[:, 512 * half:512 * (half + 1)],
                                         qth, kt_sb[hb:hb + 64, p, c0:c0 + 512])
                    nc.scalar.activation(
                        st["en"][:, 1024 * g:1024 * (g + 1)], ps[:], AF.Exp,
                        scale=1.0 / (QK8 * QK8),
                        accum_out=st["dparts"][:, g:g + 1])
                    nc.sync.dma_start_transpose(
                        st["et"][:, 8 * g:8 * (g + 1), :],
                        st["en"][:, 1024 * g:1024 * (g + 1)])

            def emit_u(h, tlo, thi):
                st = live[h]
                if tlo == 0:
                    st["ub"] = psC.tile([128, HD], DT, name="u_ps", tag="C")
                for t in range(tlo, thi):
                    nc.tensor.matmul(st["ub"][:], st["et"][:, t, :],
                                     v_sb[:, t, HD * h:HD * (h + 1)],
                                     start=(t == 0), stop=(t == LVT - 1))

            def emit_chain(h):
                st = live[h]
                dsum = smp.tile([128, 1], DT, name="dsum", tag="dsum")
                nc.vector.reduce_sum(dsum[:], st["dparts"][:],
                                     axis=mybir.AxisListType.X)
                rd = smp.tile([128, 1], DT, name="rd", tag="rd")
                nc.vector.reciprocal(rd[:], dsum[:])
                rr = smp.tile([128, 1], DT, name="rr", tag="rr")
                nc.vector.tensor_mul(rr[:], rd[:], rd[:])
                up = smp.tile([128, HD], BF, name="up", tag="up")
                # per-partition scale AP applies 1/d^2 during PSUM evacuation
                nc.scalar.activation(up[:], st["ub"][:], AF.Identity,
                                     scale=rr[:])
                st["up"] = up

            def emit_out(h):
                st = live.pop(h)
                for n in range(NCH):
                    ob = psB.tile([64, 512], DT, name="ot_ps", tag="B")
                    nc.tensor.matmul(ob[:], st["up"][:],
                                     st["en"][:, 512 * n:512 * (n + 1)])
                    ost = otsg.tile([64, 512], BF, name="ot_st", tag="ot_st")
                    nc.vector.tensor_copy(ost[:], ob[:])
                    nc.sync.dma_start(
                        ot_d[b, 64 * h:64 * (h + 1), 512 * n:512 * (n + 1)],
                        ost[:])

            for h in range(H):
                if h > 0:
                    emit_u(h - 1, 0, 16)
                emit_S(h, 0, 2)
                if h > 0:
                    emit_u(h - 1, 16, 32)
                emit_S(h, 2, 4)
                if h > 0:
                    emit_chain(h - 1)
                    emit_out(h - 1)
            emit_u(H - 1, 0, 16)
            emit_u(H - 1, 16, 32)
            emit_chain(H - 1)
            emit_out(H - 1)

            p3_scope.__exit__(None, None, None)
            # ---------- P4: final projection ----------
            p4_scope = nc.named_scope(f"p4_b{b}"); p4_scope.__enter__()
            osrc = ot_d[b].rearrange("(k p) n -> p k n", p=128)
            for c in range(NCH):
                oti = otip.tile([128, KD, 512], BF, name="oti", tag="oti")
                nc.sync.dma_start(oti[:], osrc[:, :, 512 * c:512 * (c + 1)])
                for m in range(KD):
                    ps = psA.tile([128, 512], DT, name="f_ps", tag="A")
                    for k in range(KD):
                        nc.tensor.matmul(ps[:], wo_sb[:, k, 128 * m:128 * (m + 1)],
                                         oti[:, k, :],
                                         start=(k == 0), stop=(k == KD - 1))
                    st = fosg.tile([128, 512], DT, name="fin_st", tag="fin_st")
                    nc.scalar.activation(st[:], ps[:], AF.Identity,
                                         bias=bias_sb["bo"][:, m:m + 1])
                    nc.sync.dma_start(
                        out_d[b, 128 * m:128 * (m + 1), 512 * c:512 * (c + 1)],
                        st[:])
            p4_scope.__exit__(None, None, None)

    nc.compile()
    return nc


_nc_cache = {}


def _get_program(nb=NB, with_bv=False):
    key = (nb, with_bv)
    if key not in _nc_cache:
        _nc_cache[key] = build_program(nb, with_bv)
    return _nc_cache[key]


def make_in_maps(hidden_states, text_states, Wq, bq, Wk, bk, Wv, bv, Wo, bo):
    """Host-side staging: transpose to feature-major, shard batches."""
    import ml_dtypes
    f32 = np.float32
    bf16 = ml_dtypes.bfloat16
    hs = np.asarray(hidden_states, f32)
    ts = np.asarray(text_states, f32)
    xt_all = np.ascontiguousarray(hs.transpose(0, 2, 1)).astype(bf16)  # (B,D,LV)
    # Faithful to the reference's torch-style .view: text_states (LT, B, D)
    # reinterpreted in raw memory order as (B, LT, D), then feature-major.
    tt_all = np.ascontiguousarray(
        ts.reshape(B, LT, D).transpose(0, 2, 1)).astype(bf16)
    with_bv = bool(np.any(np.asarray(bv)))
    shared = {
        "wq": np.asarray(Wq, f32).astype(bf16),
        "wk": np.asarray(Wk, f32).astype(bf16),
        "wv": np.asarray(Wv, f32).astype(bf16),
        "wo": np.asarray(Wo, f32).astype(bf16),
        "bqs8": np.ascontiguousarray(np.asarray(bq, f32) * (SCALE * QK8)),
        "bk8": np.ascontiguousarray(np.asarray(bk, f32) * QK8),
        "bo": np.ascontiguousarray(np.asarray(bo, f32)),
    }
    if with_bv:
        shared["bv"] = np.ascontiguousarray(np.asarray(bv, f32))
    in_maps = []
    for c in range(N_CORES):
        sl = slice(c * NB, (c + 1) * NB)
        in_maps.append({
            "xt": np.ascontiguousarray(xt_all[sl]),
            "tt": np.ascontiguousarray(tt_all[sl]),
            **shared,
        })
    return in_maps, with_bv


def kernel(hidden_states, text_states, Wq, bq, Wk, bk, Wv, bv, Wo, bo):
    in_maps, with_bv = make_in_maps(hidden_states, text_states, Wq, bq,
                                    Wk, bk, Wv, bv, Wo, bo)
    nc = _get_program(with_bv=with_bv)
    res = bass_utils.run_bass_kernel_spmd(nc, in_maps, list(range(N_CORES)))
    out = np.empty((B, LV, D), np.float32)
    for c in range(N_CORES):
        o = res.results[c]["out"]                                  # (NB, D, LV)
        for j in range(NB):
            out[c * NB + j] = o[j].T
    return out


# revision 6
# speedup vs baseline: 1.1928x; 1.1741x over previous
"""CLIP cross-attention kernel for 8 TRN2 NeuronCores (v2).

Math (per batch b, head h):
  Q = (T @ Wq + bq) * scale           T = text_states[:, b, :]   (128, 1024)
  K = X @ Wk + bk                     X = hidden_states[b]       (4096, 1024)
  V = X @ Wv + bv
  S = Q_h @ K_h^T                     (128, 4096)
  E = exp(S); d = rowsum(E)
  out_h = E^T @ (E @ V_h) / d^2       (4096, 64)
  final = concat_h(out_h) @ Wo + bo

Sharding: batch across 8 cores (2 batches each), weights replicated.

v2 design vs baseline:
 - K^T and V stay RESIDENT in SBUF (no DRAM round trip, no descriptor storm).
   X is streamed in 512-column chunks during the K/V projections.
 - K^T and Q^T are stored as fp8e4 (scaled by 8): S matmuls run fp8,
   kt shrinks to 32KB/partition. Verified numerically: rel err ~7.8e-3.
 - E^T is produced by the DMA xbar transpose engine (dma_start_transpose)
   from E, replacing the S^T matmul pass + second exp: saves ~135us of PE
   and ~170us of ACT per core.
 - Attention head loop is software-pipelined (emit S(h); consume(h-1))
   so the PE never waits on the ACT exp / DVE normalization chain.
 - All pools hoisted to top level; phases of consecutive batches overlap.
"""
import sys
import numpy as np

sys.path.insert(0, '/opt/trn_rl_repo')

import concourse.bass as bass          # noqa: E402
import concourse.tile as tile          # noqa: E402
from concourse import bacc, mybir      # noqa: E402
from concourse import bass_utils       # noqa: E402
from contextlib import ExitStack       # noqa: E402

DT = mybir.dt.float32
BF = mybir.dt.bfloat16
F8 = mybir.dt.float8e4
AF = mybir.ActivationFunctionType

B, LT, LV, D, H = 16, 128, 4096, 1024, 16
HD = D // H          # 64
NB = 2               # batches per core
N_CORES = 8
SCALE = HD ** -0.5
KD = D // 128        # 8
LVT = LV // 128      # 32
NCH = LV // 512      # 8
QK8 = 8.0            # fp8 storage scale for q/k


def build_program(nb=NB, with_bv=False):
    nc = bacc.Bacc("TRN2", target_bir_lowering=False, debug=False)

    xt_d = nc.dram_tensor("xt", [nb, D, LV], BF, kind="ExternalInput")
    tt_d = nc.dram_tensor("tt", [nb, D, LT], BF, kind="ExternalInput")
    w_d = {nm: nc.dram_tensor(nm, [D, D], BF, kind="ExternalInput")
           for nm in ("wq", "wk", "wv", "wo")}
    b_d = {nm: nc.dram_tensor(nm, [D], DT, kind="ExternalInput")
           for nm in ("bqs8", "bk8", "bo")}
    if with_bv:
        b_d["bv"] = nc.dram_tensor("bv", [D], DT, kind="ExternalInput")
    out_d = nc.dram_tensor("out", [nb, D, LV], DT, kind="ExternalOutput")
    ot_d = nc.dram_tensor("ot_scratch", [nb, D, LV], BF)

    with tile.TileContext(nc) as tc, ExitStack() as top:
        ep = top.enter_context
        wpool = ep(tc.tile_pool(name="wp", bufs=2))
        biasp = ep(tc.tile_pool(name="biasp", bufs=1))
        xtp = ep(tc.tile_pool(name="xtp", bufs=2))
        ktp = ep(tc.tile_pool(name="ktp", bufs=1))
        vp = ep(tc.tile_pool(name="vp", bufs=1))
        ttp = ep(tc.tile_pool(name="ttp", bufs=1))
        qtp = ep(tc.tile_pool(name="qtp", bufs=1))
        enp = ep(tc.tile_pool(name="enp", bufs=2))
        etp = ep(tc.tile_pool(name="etp", bufs=2))
        smp = ep(tc.tile_pool(name="smp", bufs=2))
        otsg = ep(tc.tile_pool(name="otsg", bufs=2))
        fosg = ep(tc.tile_pool(name="fosg", bufs=3))
        psA = ep(tc.tile_pool(name="psA", bufs=2, space="PSUM"))
        psB = ep(tc.tile_pool(name="psB", bufs=2, space="PSUM"))
        psC = ep(tc.tile_pool(name="psC", bufs=2, space="PSUM"))

        def load_weight(nm):
            t = wpool.tile([128, KD, D], BF, name=f"w_{nm}", tag="w")
            src = w_d[nm].ap().rearrange("(k p) n -> p k n", p=128)
            for k in range(KD):
                nc.sync.dma_start(t[:, k, :], src[:, k, :])
            return t

        bias_sb = {}
        for nm in b_d:
            t = biasp.tile([128, KD], DT, name=f"b_{nm}", tag=f"b_{nm}")
            nc.sync.dma_start(t[:], b_d[nm].ap().rearrange("(k p) -> p k", p=128))
            bias_sb[nm] = t

        bv_bcast = None
        if with_bv:
            bv_row = biasp.tile([1, D], DT, tag="bv_row")
            nc.sync.dma_start(bv_row[:], b_d["bv"].ap().unsqueeze(0))
            ones_row = biasp.tile([1, 128], DT, tag="ones_row")
            nc.vector.memset(ones_row[:], 1.0)
            bv_bcast = biasp.tile([128, D], DT, tag="bv_bcast")
            for g in range(2):
                pb = psB.tile([128, 512], DT, name="bv_ps", tag="B")
                nc.tensor.matmul(pb[:], ones_row[:],
                                 bv_row[:, 512 * g:512 * (g + 1)])
                nc.vector.tensor_copy(bv_bcast[:, 512 * g:512 * (g + 1)], pb[:])

        for b in range(nb):
            # ---------- P1: K^T (fp8, resident) + V (bf16, resident) ----------
            p1_scope = nc.named_scope(f"p1_b{b}"); p1_scope.__enter__()
            wk_sb = load_weight("wk")
            wv_sb = load_weight("wv")
            kt_sb = ktp.tile([128, KD, LV], F8, name="kt", tag="kt")
            v_sb = vp.tile([128, LVT, D], BF, name="v", tag="v")
            xsrc = xt_d[b].rearrange("(k p) n -> p k n", p=128)

            for cp in range(LV // 1024):
                xts = []
                for half in range(2):
                    c0 = 1024 * cp + 512 * half
                    xt_t = xtp.tile([128, KD, 512], BF, name="xt_c", tag="xt")
                    nc.sync.dma_start(xt_t[:], xsrc[:, :, c0:c0 + 512])
                    xts.append(xt_t)
                # K^T: out rows m-block, cols = this 1024-chunk
                for m in range(KD):
                    ps = psA.tile([128, 1024], DT, name="k_ps", tag="A")
                    for k in range(KD):
                        lw = wk_sb[:, k, 128 * m:128 * (m + 1)]
                        for half in range(2):
                            nc.tensor.matmul(ps[:, 512 * half:512 * (half + 1)],
                                             lw, xts[half][:, k, :],
                                             start=(k == 0), stop=(k == KD - 1))
                    nc.scalar.activation(
                        kt_sb[:, m, 1024 * cp:1024 * (cp + 1)], ps[:],
                        AF.Identity, bias=bias_sb["bk8"][:, m:m + 1], scale=QK8)
                # V: vis-tiles of this chunk pair, natural layout
                for half in range(2):
                    for tl in range(4):
                        t_abs = 8 * cp + 4 * half + tl
                        for g in range(2):
                            ps = psB.tile([128, 512], DT, name="v_ps", tag="B")
                            for k in range(KD):
                                nc.tensor.matmul(
                                    ps[:],
                                    xts[half][:, k, 128 * tl:128 * (tl + 1)],
                                    wv_sb[:, k, 512 * g:512 * (g + 1)],
                                    start=(k == 0), stop=(k == KD - 1))
                            dst = v_sb[:, t_abs, 512 * g:512 * (g + 1)]
                            if with_bv:
                                nc.vector.tensor_add(
                                    dst, ps[:], bv_bcast[:, 512 * g:512 * (g + 1)])
                            else:
                                nc.vector.tensor_copy(dst, ps[:])

            p1_scope.__exit__(None, None, None)
            # ---------- P2: Q^T (fp8) ----------
            p2_scope = nc.named_scope(f"p2_b{b}"); p2_scope.__enter__()
            tt_sb = ttp.tile([128, KD, LT], BF, name="tt", tag="tt")
            nc.sync.dma_start(tt_sb[:], tt_d[b].rearrange("(k p) t -> p k t", p=128))
            wq_sb = load_weight("wq")
            qt_sb = qtp.tile([128, KD, LT], F8, name="qt", tag="qt")
            for m in range(KD):
                ps = psC.tile([128, LT], DT, name="q_ps", tag="C")
                for k in range(KD):
                    nc.tensor.matmul(ps[:], wq_sb[:, k, 128 * m:128 * (m + 1)],
                                     tt_sb[:, k, :],
                                     start=(k == 0), stop=(k == KD - 1))
                nc.scalar.activation(qt_sb[:, m, :], ps[:], AF.Identity,
                                     bias=bias_sb["bqs8"][:, m:m + 1],
                                     scale=SCALE * QK8)

            wo_sb = load_weight("wo")   # prefetch during attention
            p2_scope.__exit__(None, None, None)
            p3_scope = nc.named_scope(f"p3_b{b}"); p3_scope.__enter__()

            # ---------- P3: attention, software-pipelined over heads ----------
            # Emission order per head h keeps the PE fed while the ACT exp /
            # DVE normalization chains for neighbouring heads complete:
            #   u(h-1)[0:16] | S(h)[g0,g1] | u(h-1)[16:32] | S(h)[g2,g3] |
            #   chain(h-1) | out(h-1) | casts+DMA(h-1)
            live = {}

            def emit_S(h, glo, ghi):
                p, hb = h // 2, 64 * (h % 2)
                qth = qt_sb[hb:hb + 64, p, :]
                if glo == 0:
                    live[h] = {
                        "en": enp.tile([128, LV], BF, name="en", tag="en"),
                        "et": etp.tile([128, LVT, 128], BF, name="et", tag="et"),
                        "dparts": smp.tile([128, 4], DT, name="dparts",
                                           tag="dparts"),
                    }
                st = live[h]
                for g in range(glo, ghi):
                    ps = psA.tile([128, 1024], DT, name="s_ps", tag="A")
                    for half in range(2):
                        c0 = 1024 * g + 512 * half
                        nc.tensor.matmul(ps[:, 512 * half:512 * (half + 1)],
                                         qth, kt_sb[hb:hb + 64, p, c0:c0 + 512])
                    nc.scalar.activation(
                        st["en"][:, 1024 * g:1024 * (g + 1)], ps[:], AF.Exp,
                        scale=1.0 / (QK8 * QK8),
                        accum_out=st["dparts"][:, g:g + 1])
                    nc.sync.dma_start_transpose(
                        st["et"][:, 8 * g:8 * (g + 1), :],
                        st["en"][:, 1024 * g:1024 * (g + 1)])

            def emit_u(h, tlo, thi):
                st = live[h]
                if tlo == 0:
                    st["ub"] = psC.tile([128, HD], DT, name="u_ps", tag="C")
                for t in range(tlo, thi):
                    nc.tensor.matmul(st["ub"][:], st["et"][:, t, :],
                                     v_sb[:, t, HD * h:HD * (h + 1)],
                                     start=(t == 0), stop=(t == LVT - 1))

            def emit_chain(h):
                st = live[h]
                dsum = smp.tile([128, 1], DT, name="dsum", tag="dsum")
                nc.vector.reduce_sum(dsum[:], st["dparts"][:],
                                     axis=mybir.AxisListType.X)
                rd = smp.tile([128, 1], DT, name="rd", tag="rd")
                nc.vector.reciprocal(rd[:], dsum[:])
                rr = smp.tile([128, 1], DT, name="rr", tag="rr")
                nc.vector.tensor_mul(rr[:], rd[:], rd[:])
                up = smp.tile([128, HD], BF, name="up", tag="up")
                # per-partition scale AP applies 1/d^2 during PSUM evacuation
                nc.scalar.activation(up[:], st["ub"][:], AF.Identity,
                                     scale=rr[:])
                st["up"] = up

            def emit_out(h):
                st = live.pop(h)
                ost = otsg.tile([64, LV], BF, name="ot_st", tag="ot_st")
                for n in range(NCH):
                    ob = psB.tile([64, 512], DT, name="ot_ps", tag="B")
                    nc.tensor.matmul(ob[:], st["up"][:],
                                     st["en"][:, 512 * n:512 * (n + 1)])
                    nc.vector.tensor_copy(ost[:, 512 * n:512 * (n + 1)], ob[:])
                # one DMA per head keeps the sync queue free for transposes
                nc.sync.dma_start(ot_d[b, 64 * h:64 * (h + 1), :], ost[:])

            for h in range(H):
                if h > 0:
                    emit_u(h - 1, 0, 16)
                emit_S(h, 0, 2)
                if h > 0:
                    emit_u(h - 1, 16, 32)
                emit_S(h, 2, 4)
                if h > 0:
                    emit_chain(h - 1)
                    emit_out(h - 1)
            emit_u(H - 1, 0, 16)
            emit_u(H - 1, 16, 32)
            emit_chain(H - 1)
            emit_out(H - 1)

            p3_scope.__exit__(None, None, None)
            # ---------- P4: final projection ----------
            p4_scope = nc.named_scope(f"p4_b{b}"); p4_scope.__enter__()
            osrc = ot_d[b].rearrange("(k p) n -> p k n", p=128)
            for c in range(NCH):
                oti = xtp.tile([128, KD, 512], BF, name="oti", tag="xt")
                nc.sync.dma_start(oti[:], osrc[:, :, 512 * c:512 * (c + 1)])
                for m in range(KD):
                    ps = psA.tile([128, 512], DT, name="f_ps", tag="A")
                    for k in range(KD):
                        nc.tensor.matmul(ps[:], wo_sb[:, k, 128 * m:128 * (m + 1)],
                                         oti[:, k, :],
                                         start=(k == 0), stop=(k == KD - 1))
                    st = fosg.tile([128, 512], DT, name="fin_st", tag="fin_st")
                    nc.scalar.activation(st[:], ps[:], AF.Identity,
                                         bias=bias_sb["bo"][:, m:m + 1])
                    nc.sync.dma_start(
                        out_d[b, 128 * m:128 * (m + 1), 512 * c:512 * (c + 1)],
                        st[:])
            p4_scope.__exit__(None, None, None)

    nc.compile()
    return nc


_nc_cache = {}


def _get_program(nb=NB, with_bv=False):
    key = (nb, with_bv)
    if key not in _nc_cache:
        _nc_cache[key] = build_program(nb, with_bv)
    return _nc_cache[key]


def make_in_maps(hidden_states, text_states, Wq, bq, Wk, bk, Wv, bv, Wo, bo):
    """Host-side staging: transpose to feature-major, shard batches."""
    import ml_dtypes
    f32 = np.float32
    bf16 = ml_dtypes.bfloat16
    hs = np.asarray(hidden_states, f32)
    ts = np.asarray(text_states, f32)
    xt_all = np.ascontiguousarray(hs.transpose(0, 2, 1)).astype(bf16)  # (B,D,LV)
    # Faithful to the reference's torch-style .view: text_states (LT, B, D)
    # reinterpreted in raw memory order as (B, LT, D), then feature-major.
    tt_all = np.ascontiguousarray(
        ts.reshape(B, LT, D).transpose(0, 2, 1)).astype(bf16)
    with_bv = bool(np.any(np.asarray(bv)))
    shared = {
        "wq": np.asarray(Wq, f32).astype(bf16),
        "wk": np.asarray(Wk, f32).astype(bf16),
        "wv": np.asarray(Wv, f32).astype(bf16),
        "wo": np.asarray(Wo, f32).astype(bf16),
        "bqs8": np.ascontiguousarray(np.asarray(bq, f32) * (SCALE * QK8)),
        "bk8": np.ascontiguousarray(np.asarray(bk, f32) * QK8),
        "bo": np.ascontiguousarray(np.asarray(bo, f32)),
    }
    if with_bv:
        shared["bv"] = np.ascontiguousarray(np.asarray(bv, f32))
    in_maps = []
    for c in range(N_CORES):
        sl = slice(c * NB, (c + 1) * NB)
        in_maps.append({
            "xt": np.ascontiguousarray(xt_all[sl]),
            "tt": np.ascontiguousarray(tt_all[sl]),
            **shared,
        })
    return in_maps, with_bv


def kernel(hidden_states, text_states, Wq, bq, Wk, bk, Wv, bv, Wo, bo):
    in_maps, with_bv = make_in_maps(hidden_states, text_states, Wq, bq,
                                    Wk, bk, Wv, bv, Wo, bo)
    nc = _get_program(with_bv=with_bv)
    res = bass_utils.run_bass_kernel_spmd(nc, in_maps, list(range(N_CORES)))
    out = np.empty((B, LV, D), np.float32)
    for c in range(N_CORES):
        o = res.results[c]["out"]                                  # (NB, D, LV)
        for j in range(NB):
            out[c * NB + j] = o[j].T
    return out


# revision 9
# speedup vs baseline: 1.2171x; 1.0204x over previous
"""CLIP cross-attention kernel for 8 TRN2 NeuronCores (v2).

Math (per batch b, head h):
  Q = (T @ Wq + bq) * scale           T = text_states[:, b, :]   (128, 1024)
  K = X @ Wk + bk                     X = hidden_states[b]       (4096, 1024)
  V = X @ Wv + bv
  S = Q_h @ K_h^T                     (128, 4096)
  E = exp(S); d = rowsum(E)
  out_h = E^T @ (E @ V_h) / d^2       (4096, 64)
  final = concat_h(out_h) @ Wo + bo

Sharding: batch across 8 cores (2 batches each), weights replicated.

v2 design vs baseline:
 - K^T and V stay RESIDENT in SBUF (no DRAM round trip, no descriptor storm).
   X is streamed in 512-column chunks during the K/V projections.
 - K^T and Q^T are stored as fp8e4 (scaled by 8): S matmuls run fp8,
   kt shrinks to 32KB/partition. Verified numerically: rel err ~7.8e-3.
 - E^T is produced by the DMA xbar transpose engine (dma_start_transpose)
   from E, replacing the S^T matmul pass + second exp: saves ~135us of PE
   and ~170us of ACT per core.
 - Attention head loop is software-pipelined (emit S(h); consume(h-1))
   so the PE never waits on the ACT exp / DVE normalization chain.
 - All pools hoisted to top level; phases of consecutive batches overlap.
"""
import sys
import numpy as np

sys.path.insert(0, '/opt/trn_rl_repo')

import concourse.bass as bass          # noqa: E402
import concourse.tile as tile          # noqa: E402
from concourse import bacc, mybir      # noqa: E402
from concourse import bass_utils       # noqa: E402
from contextlib import ExitStack       # noqa: E402

DT = mybir.dt.float32
BF = mybir.dt.bfloat16
F8 = mybir.dt.float8e4
AF = mybir.ActivationFunctionType

B, LT, LV, D, H = 16, 128, 4096, 1024, 16
HD = D // H          # 64
NB = 2               # batches per core
N_CORES = 8
SCALE = HD ** -0.5
KD = D // 128        # 8
LVT = LV // 128      # 32
NCH = LV // 512      # 8
QK8 = 8.0            # fp8 storage scale for q/k


def build_program(nb=NB, with_bv=False):
    nc = bacc.Bacc("TRN2", target_bir_lowering=False, debug=False)

    xt_d = nc.dram_tensor("xt", [nb, D, LV], BF, kind="ExternalInput")
    tt_d = nc.dram_tensor("tt", [nb, D, LT], BF, kind="ExternalInput")
    w_d = {nm: nc.dram_tensor(nm, [D, D], BF, kind="ExternalInput")
           for nm in ("wq", "wk", "wv", "wo")}
    b_d = {nm: nc.dram_tensor(nm, [D], DT, kind="ExternalInput")
           for nm in ("bqs8", "bk8", "bo")}
    if with_bv:
        b_d["bv"] = nc.dram_tensor("bv", [D], DT, kind="ExternalInput")
    out_d = nc.dram_tensor("out", [nb, D, LV], DT, kind="ExternalOutput")
    ot_d = nc.dram_tensor("ot_scratch", [nb, D, LV], BF)

    with tile.TileContext(nc) as tc, ExitStack() as top:
        ep = top.enter_context
        wpool = ep(tc.tile_pool(name="wp", bufs=2))
        biasp = ep(tc.tile_pool(name="biasp", bufs=1))
        xtp = ep(tc.tile_pool(name="xtp", bufs=2))
        ktp = ep(tc.tile_pool(name="ktp", bufs=1))
        vp = ep(tc.tile_pool(name="vp", bufs=1))
        ttp = ep(tc.tile_pool(name="ttp", bufs=1))
        qtp = ep(tc.tile_pool(name="qtp", bufs=1))
        enp = ep(tc.tile_pool(name="enp", bufs=2))
        etp = ep(tc.tile_pool(name="etp", bufs=2))
        smp = ep(tc.tile_pool(name="smp", bufs=2))
        otsg = ep(tc.tile_pool(name="otsg", bufs=2))
        fosg = ep(tc.tile_pool(name="fosg", bufs=3))
        psA = ep(tc.tile_pool(name="psA", bufs=2, space="PSUM"))
        psB = ep(tc.tile_pool(name="psB", bufs=2, space="PSUM"))
        psC = ep(tc.tile_pool(name="psC", bufs=2, space="PSUM"))

        def load_weight(nm):
            t = wpool.tile([128, KD, D], BF, name=f"w_{nm}", tag="w")
            src = w_d[nm].ap().rearrange("(k p) n -> p k n", p=128)
            for k in range(KD):
                nc.sync.dma_start(t[:, k, :], src[:, k, :])
            return t

        bias_sb = {}
        for nm in b_d:
            t = biasp.tile([128, KD], DT, name=f"b_{nm}", tag=f"b_{nm}")
            nc.sync.dma_start(t[:], b_d[nm].ap().rearrange("(k p) -> p k", p=128))
            bias_sb[nm] = t

        bv_bcast = None
        if with_bv:
            bv_row = biasp.tile([1, D], DT, tag="bv_row")
            nc.sync.dma_start(bv_row[:], b_d["bv"].ap().unsqueeze(0))
            ones_row = biasp.tile([1, 128], DT, tag="ones_row")
            nc.vector.memset(ones_row[:], 1.0)
            bv_bcast = biasp.tile([128, D], DT, tag="bv_bcast")
            for g in range(2):
                pb = psB.tile([128, 512], DT, name="bv_ps", tag="B")
                nc.tensor.matmul(pb[:], ones_row[:],
                                 bv_row[:, 512 * g:512 * (g + 1)])
                nc.vector.tensor_copy(bv_bcast[:, 512 * g:512 * (g + 1)], pb[:])

        for b in range(nb):
            # ---------- P1: K^T (fp8, resident) + V (bf16, resident) ----------
            p1_scope = nc.named_scope(f"p1_b{b}"); p1_scope.__enter__()
            wk_sb = load_weight("wk")
            wv_sb = load_weight("wv")
            kt_sb = ktp.tile([128, KD, LV], F8, name="kt", tag="kt")
            v_sb = vp.tile([128, LVT, D], BF, name="v", tag="v")
            xsrc = xt_d[b].rearrange("(k p) n -> p k n", p=128)

            for cp in range(LV // 1024):
                xts = []
                for half in range(2):
                    c0 = 1024 * cp + 512 * half
                    xt_t = xtp.tile([128, KD, 512], BF, name="xt_c", tag="xt")
                    nc.sync.dma_start(xt_t[:], xsrc[:, :, c0:c0 + 512])
                    xts.append(xt_t)
                # K^T: out rows m-block, cols = this 1024-chunk
                for m in range(KD):
                    ps = psA.tile([128, 1024], DT, name="k_ps", tag="A")
                    for k in range(KD):
                        lw = wk_sb[:, k, 128 * m:128 * (m + 1)]
                        for half in range(2):
                            nc.tensor.matmul(ps[:, 512 * half:512 * (half + 1)],
                                             lw, xts[half][:, k, :],
                                             start=(k == 0), stop=(k == KD - 1))
                    nc.scalar.activation(
                        kt_sb[:, m, 1024 * cp:1024 * (cp + 1)], ps[:],
                        AF.Identity, bias=bias_sb["bk8"][:, m:m + 1], scale=QK8)
                # V cols 0:512 (heads 0-7) here; cols 512:1024 are computed
                # inside the attention loop as PE filler (emit_vg1).
                for half in range(2):
                    for tl in range(4):
                        t_abs = 8 * cp + 4 * half + tl
                        ps = psB.tile([128, 512], DT, name="v_ps", tag="B")
                        for k in range(KD):
                            nc.tensor.matmul(
                                ps[:],
                                xts[half][:, k, 128 * tl:128 * (tl + 1)],
                                wv_sb[:, k, 0:512],
                                start=(k == 0), stop=(k == KD - 1))
                        dst = v_sb[:, t_abs, 0:512]
                        if with_bv:
                            nc.vector.tensor_add(
                                dst, ps[:], bv_bcast[:, 0:512])
                        else:
                            nc.vector.tensor_copy(dst, ps[:])

            p1_scope.__exit__(None, None, None)
            # ---------- P2: Q^T (fp8) ----------
            p2_scope = nc.named_scope(f"p2_b{b}"); p2_scope.__enter__()
            tt_sb = ttp.tile([128, KD, LT], BF, name="tt", tag="tt")
            nc.sync.dma_start(tt_sb[:], tt_d[b].rearrange("(k p) t -> p k t", p=128))
            wq_sb = load_weight("wq")
            qt_sb = qtp.tile([128, KD, LT], F8, name="qt", tag="qt")
            for m in range(KD):
                ps = psC.tile([128, LT], DT, name="q_ps", tag="C")
                for k in range(KD):
                    nc.tensor.matmul(ps[:], wq_sb[:, k, 128 * m:128 * (m + 1)],
                                     tt_sb[:, k, :],
                                     start=(k == 0), stop=(k == KD - 1))
                nc.scalar.activation(qt_sb[:, m, :], ps[:], AF.Identity,
                                     bias=bias_sb["bqs8"][:, m:m + 1],
                                     scale=SCALE * QK8)

            wo_sb = load_weight("wo")   # prefetch during attention
            p2_scope.__exit__(None, None, None)
            p3_scope = nc.named_scope(f"p3_b{b}"); p3_scope.__enter__()

            # ---------- P3: attention, software-pipelined over heads ----------
            # Emission order per head h keeps the PE fed while the ACT exp /
            # DVE normalization chains for neighbouring heads complete:
            #   u(h-1)[0:16] | S(h)[g0,g1] | u(h-1)[16:32] | S(h)[g2,g3] |
            #   chain(h-1) | out(h-1) | casts+DMA(h-1)
            live = {}

            def emit_S(h, glo, ghi):
                p, hb = h // 2, 64 * (h % 2)
                qth = qt_sb[hb:hb + 64, p, :]
                if glo == 0:
                    live[h] = {
                        "en": enp.tile([128, LV], BF, name="en", tag="en"),
                        "et": etp.tile([128, LVT, 128], BF, name="et", tag="et"),
                        "dparts": smp.tile([128, 4], DT, name="dparts",
                                           tag="dparts"),
                    }
                st = live[h]
                for g in range(glo, ghi):
                    ps = psA.tile([128, 1024], DT, name="s_ps", tag="A")
                    for half in range(2):
                        c0 = 1024 * g + 512 * half
                        nc.tensor.matmul(ps[:, 512 * half:512 * (half + 1)],
                                         qth, kt_sb[hb:hb + 64, p, c0:c0 + 512])
                    nc.scalar.activation(
                        st["en"][:, 1024 * g:1024 * (g + 1)], ps[:], AF.Exp,
                        scale=1.0 / (QK8 * QK8),
                        accum_out=st["dparts"][:, g:g + 1])
                    nc.sync.dma_start_transpose(
                        st["et"][:, 8 * g:8 * (g + 1), :],
                        st["en"][:, 1024 * g:1024 * (g + 1)])

            def emit_u(h, tlo, thi):
                st = live[h]
                if tlo == 0:
                    st["ub"] = psC.tile([128, HD], DT, name="u_ps", tag="C")
                for t in range(tlo, thi):
                    nc.tensor.matmul(st["ub"][:], st["et"][:, t, :],
                                     v_sb[:, t, HD * h:HD * (h + 1)],
                                     start=(t == 0), stop=(t == LVT - 1))

            def emit_chain(h):
                st = live[h]
                dsum = smp.tile([128, 1], DT, name="dsum", tag="dsum")
                nc.vector.reduce_sum(dsum[:], st["dparts"][:],
                                     axis=mybir.AxisListType.X)
                rd = smp.tile([128, 1], DT, name="rd", tag="rd")
                nc.vector.reciprocal(rd[:], dsum[:])
                rr = smp.tile([128, 1], DT, name="rr", tag="rr")
                nc.vector.tensor_mul(rr[:], rd[:], rd[:])
                up = smp.tile([128, HD], BF, name="up", tag="up")
                # per-partition scale AP applies 1/d^2 during PSUM evacuation
                nc.scalar.activation(up[:], st["ub"][:], AF.Identity,
                                     scale=rr[:])
                st["up"] = up

            def emit_vg1(h, tlo, thi):
                # V columns 512:1024 (heads 8-15) for vis-tiles 4h+tlo..4h+thi,
                # emitted inside the attention loop as dense PE filler.
                xtb = vg1_chunks[h]
                for tl in range(tlo, thi):
                    t_abs = 4 * h + tl
                    ps = psB.tile([128, 512], DT, name="v_ps1", tag="B")
                    for k in range(KD):
                        nc.tensor.matmul(
                            ps[:], xtb[:, k, 128 * tl:128 * (tl + 1)],
                            wv_sb[:, k, 512:1024],
                            start=(k == 0), stop=(k == KD - 1))
                    dst = v_sb[:, t_abs, 512:1024]
                    if with_bv:
                        nc.vector.tensor_add(dst, ps[:], bv_bcast[:, 512:1024])
                    elif tl % 2 == 0:
                        nc.scalar.activation(dst, ps[:], AF.Identity)
                    else:
                        nc.vector.tensor_copy(dst, ps[:])

            def emit_out(h):
                st = live.pop(h)
                ost = otsg.tile([64, LV], BF, name="ot_st", tag="ot_st")
                for n in range(NCH):
                    obp = psB if n % 2 == 0 else psC
                    ob = obp.tile([64, 512], DT, name="ot_ps",
                                  tag="B" if n % 2 == 0 else "C")
                    nc.tensor.matmul(ob[:], st["up"][:],
                                     st["en"][:, 512 * n:512 * (n + 1)])
                    nc.vector.tensor_copy(ost[:, 512 * n:512 * (n + 1)], ob[:])
                    if h < 8 and n % 2 == 1:
                        emit_vg1(h, n // 2, n // 2 + 1)
                # one DMA per head keeps the sync queue free for transposes
                nc.sync.dma_start(ot_d[b, 64 * h:64 * (h + 1), :], ost[:])

            vg1_chunks = {}
            for h in range(H):
                if h < 8:
                    # reload xt chunk h for the deferred V columns
                    xtb = xtp.tile([128, KD, 512], BF, name="xt_v1", tag="xt")
                    nc.sync.dma_start(xtb[:], xsrc[:, :, 512 * h:512 * (h + 1)])
                    vg1_chunks[h] = xtb
                if h > 0:
                    emit_u(h - 1, 0, 16)
                emit_S(h, 0, 2)
                if h > 0:
                    emit_u(h - 1, 16, 32)
                emit_S(h, 2, 4)
                if h > 0:
                    emit_chain(h - 1)
                    emit_out(h - 1)
            emit_u(H - 1, 0, 16)
            emit_u(H - 1, 16, 32)
            emit_chain(H - 1)
            emit_out(H - 1)

            p3_scope.__exit__(None, None, None)
            # ---------- P4: final projection ----------
            p4_scope = nc.named_scope(f"p4_b{b}"); p4_scope.__enter__()
            osrc = ot_d[b].rearrange("(k p) n -> p k n", p=128)
            for c in range(NCH):
                oti = xtp.tile([128, KD, 512], BF, name="oti", tag="xt")
                nc.sync.dma_start(oti[:], osrc[:, :, 512 * c:512 * (c + 1)])
                for m in range(KD):
                    ps = psA.tile([128, 512], DT, name="f_ps", tag="A")
                    for k in range(KD):
                        nc.tensor.matmul(ps[:], wo_sb[:, k, 128 * m:128 * (m + 1)],
                                         oti[:, k, :],
                                         start=(k == 0), stop=(k == KD - 1))
                    st = fosg.tile([128, 512], DT, name="fin_st", tag="fin_st")
                    nc.scalar.activation(st[:], ps[:], AF.Identity,
                                         bias=bias_sb["bo"][:, m:m + 1])
                    nc.sync.dma_start(
                        out_d[b, 128 * m:128 * (m + 1), 512 * c:512 * (c + 1)],
                        st[:])
            p4_scope.__exit__(None, None, None)

    nc.compile()
    return nc


_nc_cache = {}


def _get_program(nb=NB, with_bv=False):
    key = (nb, with_bv)
    if key not in _nc_cache:
        _nc_cache[key] = build_program(nb, with_bv)
    return _nc_cache[key]


def make_in_maps(hidden_states, text_states, Wq, bq, Wk, bk, Wv, bv, Wo, bo):
    """Host-side staging: transpose to feature-major, shard batches."""
    import ml_dtypes
    f32 = np.float32
    bf16 = ml_dtypes.bfloat16
    hs = np.asarray(hidden_states, f32)
    ts = np.asarray(text_states, f32)
    xt_all = np.ascontiguousarray(hs.transpose(0, 2, 1)).astype(bf16)  # (B,D,LV)
    # Faithful to the reference's torch-style .view: text_states (LT, B, D)
    # reinterpreted in raw memory order as (B, LT, D), then feature-major.
    tt_all = np.ascontiguousarray(
        ts.reshape(B, LT, D).transpose(0, 2, 1)).astype(bf16)
    with_bv = bool(np.any(np.asarray(bv)))
    shared = {
        "wq": np.asarray(Wq, f32).astype(bf16),
        "wk": np.asarray(Wk, f32).astype(bf16),
        "wv": np.asarray(Wv, f32).astype(bf16),
        "wo": np.asarray(Wo, f32).astype(bf16),
        "bqs8": np.ascontiguousarray(np.asarray(bq, f32) * (SCALE * QK8)),
        "bk8": np.ascontiguousarray(np.asarray(bk, f32) * QK8),
        "bo": np.ascontiguousarray(np.asarray(bo, f32)),
    }
    if with_bv:
        shared["bv"] = np.ascontiguousarray(np.asarray(bv, f32))
    in_maps = []
    for c in range(N_CORES):
        sl = slice(c * NB, (c + 1) * NB)
        in_maps.append({
            "xt": np.ascontiguousarray(xt_all[sl]),
            "tt": np.ascontiguousarray(tt_all[sl]),
            **shared,
        })
    return in_maps, with_bv


def kernel(hidden_states, text_states, Wq, bq, Wk, bk, Wv, bv, Wo, bo):
    in_maps, with_bv = make_in_maps(hidden_states, text_states, Wq, bq,
                                    Wk, bk, Wv, bv, Wo, bo)
    nc = _get_program(with_bv=with_bv)
    res = bass_utils.run_bass_kernel_spmd(nc, in_maps, list(range(N_CORES)))
    out = np.empty((B, LV, D), np.float32)
    for c in range(N_CORES):
        o = res.results[c]["out"]                                  # (NB, D, LV)
        for j in range(NB):
            out[c * NB + j] = o[j].T
    return out


# revision 11
# speedup vs baseline: 1.2172x; 1.0000x over previous
"""CLIP cross-attention kernel for 8 TRN2 NeuronCores (v2).

Math (per batch b, head h):
  Q = (T @ Wq + bq) * scale           T = text_states[:, b, :]   (128, 1024)
  K = X @ Wk + bk                     X = hidden_states[b]       (4096, 1024)
  V = X @ Wv + bv
  S = Q_h @ K_h^T                     (128, 4096)
  E = exp(S); d = rowsum(E)
  out_h = E^T @ (E @ V_h) / d^2       (4096, 64)
  final = concat_h(out_h) @ Wo + bo

Sharding: batch across 8 cores (2 batches each), weights replicated.

v2 design vs baseline:
 - K^T and V stay RESIDENT in SBUF (no DRAM round trip, no descriptor storm).
   X is streamed in 512-column chunks during the K/V projections.
 - K^T and Q^T are stored as fp8e4 (scaled by 8): S matmuls run fp8,
   kt shrinks to 32KB/partition. Verified numerically: rel err ~7.8e-3.
 - E^T is produced by the DMA xbar transpose engine (dma_start_transpose)
   from E, replacing the S^T matmul pass + second exp: saves ~135us of PE
   and ~170us of ACT per core.
 - Attention head loop is software-pipelined (emit S(h); consume(h-1))
   so the PE never waits on the ACT exp / DVE normalization chain.
 - All pools hoisted to top level; phases of consecutive batches overlap.
"""
import sys
import numpy as np

sys.path.insert(0, '/opt/trn_rl_repo')

import concourse.bass as bass          # noqa: E402
import concourse.tile as tile          # noqa: E402
from concourse import bacc, mybir      # noqa: E402
from concourse import bass_utils       # noqa: E402
from contextlib import ExitStack       # noqa: E402

DT = mybir.dt.float32
BF = mybir.dt.bfloat16
F8 = mybir.dt.float8e4
AF = mybir.ActivationFunctionType

B, LT, LV, D, H = 16, 128, 4096, 1024, 16
HD = D // H          # 64
NB = 2               # batches per core
N_CORES = 8
SCALE = HD ** -0.5
KD = D // 128        # 8
LVT = LV // 128      # 32
NCH = LV // 512      # 8
QK8 = 8.0            # fp8 storage scale for q/k


def build_program(nb=NB, with_bv=False):
    nc = bacc.Bacc("TRN2", target_bir_lowering=False, debug=False)

    xt_d = nc.dram_tensor("xt", [nb, D, LV], BF, kind="ExternalInput")
    tt_d = nc.dram_tensor("tt", [nb, D, LT], BF, kind="ExternalInput")
    w_d = {nm: nc.dram_tensor(nm, [D, D], BF, kind="ExternalInput")
           for nm in ("wq", "wk", "wv", "wo")}
    b_d = {nm: nc.dram_tensor(nm, [D], DT, kind="ExternalInput")
           for nm in ("bqs8", "bk8", "bo")}
    if with_bv:
        b_d["bv"] = nc.dram_tensor("bv", [D], DT, kind="ExternalInput")
    out_d = nc.dram_tensor("out", [nb, D, LV], DT, kind="ExternalOutput")
    ot_d = nc.dram_tensor("ot_scratch", [nb, D, LV], BF)

    with tile.TileContext(nc) as tc, ExitStack() as top:
        ep = top.enter_context
        wpool = ep(tc.tile_pool(name="wp", bufs=2))
        biasp = ep(tc.tile_pool(name="biasp", bufs=1))
        xtp = ep(tc.tile_pool(name="xtp", bufs=2))
        ktp = ep(tc.tile_pool(name="ktp", bufs=1))
        vp = ep(tc.tile_pool(name="vp", bufs=1))
        ttp = ep(tc.tile_pool(name="ttp", bufs=1))
        qtp = ep(tc.tile_pool(name="qtp", bufs=1))
        enp = ep(tc.tile_pool(name="enp", bufs=2))
        etp = ep(tc.tile_pool(name="etp", bufs=2))
        smp = ep(tc.tile_pool(name="smp", bufs=2))
        otsg = ep(tc.tile_pool(name="otsg", bufs=2))
        fosg = ep(tc.tile_pool(name="fosg", bufs=3))
        psA = ep(tc.tile_pool(name="psA", bufs=2, space="PSUM"))
        psB = ep(tc.tile_pool(name="psB", bufs=2, space="PSUM"))
        psC = ep(tc.tile_pool(name="psC", bufs=2, space="PSUM"))

        def load_weight(nm):
            t = wpool.tile([128, KD, D], BF, name=f"w_{nm}", tag="w")
            src = w_d[nm].ap().rearrange("(k p) n -> p k n", p=128)
            for k in range(KD):
                nc.sync.dma_start(t[:, k, :], src[:, k, :])
            return t

        bias_sb = {}
        for nm in b_d:
            t = biasp.tile([128, KD], DT, name=f"b_{nm}", tag=f"b_{nm}")
            nc.sync.dma_start(t[:], b_d[nm].ap().rearrange("(k p) -> p k", p=128))
            bias_sb[nm] = t

        bv_bcast = None
        if with_bv:
            bv_row = biasp.tile([1, D], DT, tag="bv_row")
            nc.sync.dma_start(bv_row[:], b_d["bv"].ap().unsqueeze(0))
            ones_row = biasp.tile([1, 128], DT, tag="ones_row")
            nc.vector.memset(ones_row[:], 1.0)
            bv_bcast = biasp.tile([128, D], DT, tag="bv_bcast")
            for g in range(2):
                pb = psB.tile([128, 512], DT, name="bv_ps", tag="B")
                nc.tensor.matmul(pb[:], ones_row[:],
                                 bv_row[:, 512 * g:512 * (g + 1)])
                nc.vector.tensor_copy(bv_bcast[:, 512 * g:512 * (g + 1)], pb[:])

        for b in range(nb):
            # ---------- P1: K^T (fp8, resident) + V (bf16, resident) ----------
            p1_scope = nc.named_scope(f"p1_b{b}"); p1_scope.__enter__()
            wk_sb = load_weight("wk")
            wv_sb = load_weight("wv")
            kt_sb = ktp.tile([128, KD, LV], F8, name="kt", tag="kt")
            v_sb = vp.tile([128, LVT, D], BF, name="v", tag="v")
            xsrc = xt_d[b].rearrange("(k p) n -> p k n", p=128)

            for cp in range(LV // 1024):
                xts = []
                for half in range(2):
                    c0 = 1024 * cp + 512 * half
                    xt_t = xtp.tile([128, KD, 512], BF, name="xt_c", tag="xt")
                    nc.sync.dma_start(xt_t[:], xsrc[:, :, c0:c0 + 512])
                    xts.append(xt_t)
                # K^T: out rows m-block, cols = this 1024-chunk
                for m in range(KD):
                    ps = psA.tile([128, 1024], DT, name="k_ps", tag="A")
                    for k in range(KD):
                        lw = wk_sb[:, k, 128 * m:128 * (m + 1)]
                        for half in range(2):
                            nc.tensor.matmul(ps[:, 512 * half:512 * (half + 1)],
                                             lw, xts[half][:, k, :],
                                             start=(k == 0), stop=(k == KD - 1))
                    nc.scalar.activation(
                        kt_sb[:, m, 1024 * cp:1024 * (cp + 1)], ps[:],
                        AF.Identity, bias=bias_sb["bk8"][:, m:m + 1], scale=QK8)
                # V cols 0:512 (heads 0-7) here; cols 512:1024 are computed
                # inside the attention loop as PE filler (emit_vg1).
                for half in range(2):
                    for tl in range(4):
                        t_abs = 8 * cp + 4 * half + tl
                        ps = psB.tile([128, 512], DT, name="v_ps", tag="B")
                        for k in range(KD):
                            nc.tensor.matmul(
                                ps[:],
                                xts[half][:, k, 128 * tl:128 * (tl + 1)],
                                wv_sb[:, k, 0:512],
                                start=(k == 0), stop=(k == KD - 1))
                        dst = v_sb[:, t_abs, 0:512]
                        if with_bv:
                            nc.vector.tensor_add(
                                dst, ps[:], bv_bcast[:, 0:512])
                        else:
                            nc.vector.tensor_copy(dst, ps[:])

            p1_scope.__exit__(None, None, None)
            # ---------- P2: Q^T (fp8) ----------
            p2_scope = nc.named_scope(f"p2_b{b}"); p2_scope.__enter__()
            tt_sb = ttp.tile([128, KD, LT], BF, name="tt", tag="tt")
            nc.sync.dma_start(tt_sb[:], tt_d[b].rearrange("(k p) t -> p k t", p=128))
            wq_sb = load_weight("wq")
            qt_sb = qtp.tile([128, KD, LT], F8, name="qt", tag="qt")
            for m in range(KD):
                ps = psC.tile([128, LT], DT, name="q_ps", tag="C")
                for k in range(KD):
                    nc.tensor.matmul(ps[:], wq_sb[:, k, 128 * m:128 * (m + 1)],
                                     tt_sb[:, k, :],
                                     start=(k == 0), stop=(k == KD - 1))
                nc.scalar.activation(qt_sb[:, m, :], ps[:], AF.Identity,
                                     bias=bias_sb["bqs8"][:, m:m + 1],
                                     scale=SCALE * QK8)

            wo_sb = load_weight("wo")   # prefetch during attention
            p2_scope.__exit__(None, None, None)
            p3_scope = nc.named_scope(f"p3_b{b}"); p3_scope.__enter__()

            # ---------- P3: attention, software-pipelined over heads ----------
            # Emission order per head h keeps the PE fed while the ACT exp /
            # DVE normalization chains for neighbouring heads complete:
            #   u(h-1)[0:16] | S(h)[g0,g1] | u(h-1)[16:32] | S(h)[g2,g3] |
            #   chain(h-1) | out(h-1) | casts+DMA(h-1)
            live = {}

            def emit_S(h, glo, ghi):
                p, hb = h // 2, 64 * (h % 2)
                qth = qt_sb[hb:hb + 64, p, :]
                if glo == 0:
                    live[h] = {
                        "en": enp.tile([128, LV], BF, name="en", tag="en"),
                        "et": etp.tile([128, LVT, 128], BF, name="et", tag="et"),
                        "dparts": smp.tile([128, 4], DT, name="dparts",
                                           tag="dparts"),
                    }
                st = live[h]
                for g in range(glo, ghi):
                    ps = psA.tile([128, 1024], DT, name="s_ps", tag="A")
                    for half in range(2):
                        c0 = 1024 * g + 512 * half
                        nc.tensor.matmul(ps[:, 512 * half:512 * (half + 1)],
                                         qth, kt_sb[hb:hb + 64, p, c0:c0 + 512])
                    nc.scalar.activation(
                        st["en"][:, 1024 * g:1024 * (g + 1)], ps[:], AF.Exp,
                        scale=1.0 / (QK8 * QK8),
                        accum_out=st["dparts"][:, g:g + 1])
                    nc.sync.dma_start_transpose(
                        st["et"][:, 8 * g:8 * (g + 1), :],
                        st["en"][:, 1024 * g:1024 * (g + 1)])

            def emit_u(h, tlo, thi):
                st = live[h]
                if tlo == 0:
                    st["ub"] = psC.tile([128, HD], DT, name="u_ps", tag="C")
                for t in range(tlo, thi):
                    nc.tensor.matmul(st["ub"][:], st["et"][:, t, :],
                                     v_sb[:, t, HD * h:HD * (h + 1)],
                                     start=(t == 0), stop=(t == LVT - 1))

            def emit_chain(h):
                st = live[h]
                dsum = smp.tile([128, 1], DT, name="dsum", tag="dsum")
                nc.vector.reduce_sum(dsum[:], st["dparts"][:],
                                     axis=mybir.AxisListType.X)
                rd = smp.tile([128, 1], DT, name="rd", tag="rd")
                nc.vector.reciprocal(rd[:], dsum[:])
                rr = smp.tile([128, 1], DT, name="rr", tag="rr")
                nc.vector.tensor_mul(rr[:], rd[:], rd[:])
                up = smp.tile([128, HD], BF, name="up", tag="up")
                # per-partition scale AP applies 1/d^2 during PSUM evacuation
                nc.scalar.activation(up[:], st["ub"][:], AF.Identity,
                                     scale=rr[:])
                st["up"] = up

            def emit_vg1(h, tlo, thi):
                # V columns 512:1024 (heads 8-15) for vis-tiles 4h+tlo..4h+thi,
                # emitted inside the attention loop as dense PE filler.
                xtb = vg1_chunks[h]
                for tl in range(tlo, thi):
                    t_abs = 4 * h + tl
                    ps = psB.tile([128, 512], DT, name="v_ps1", tag="B")
                    for k in range(KD):
                        nc.tensor.matmul(
                            ps[:], xtb[:, k, 128 * tl:128 * (tl + 1)],
                            wv_sb[:, k, 512:1024],
                            start=(k == 0), stop=(k == KD - 1))
                    dst = v_sb[:, t_abs, 512:1024]
                    if with_bv:
                        nc.vector.tensor_add(dst, ps[:], bv_bcast[:, 512:1024])
                    elif tl % 2 == 0:
                        nc.scalar.activation(dst, ps[:], AF.Identity)
                    else:
                        nc.vector.tensor_copy(dst, ps[:])

            def emit_out(h):
                st = live.pop(h)
                ost = otsg.tile([64, LV], BF, name="ot_st", tag="ot_st")
                for n in range(NCH):
                    obp = psB if n % 2 == 0 else psC
                    ob = obp.tile([64, 512], DT, name="ot_ps",
                                  tag="B" if n % 2 == 0 else "C")
                    nc.tensor.matmul(ob[:], st["up"][:],
                                     st["en"][:, 512 * n:512 * (n + 1)])
                    nc.vector.tensor_copy(ost[:, 512 * n:512 * (n + 1)], ob[:])
                    if h < 8 and n % 2 == 1:
                        emit_vg1(h, n // 2, n // 2 + 1)
                # one DMA per head keeps the sync queue free for transposes
                nc.sync.dma_start(ot_d[b, 64 * h:64 * (h + 1), :], ost[:])

            vg1_chunks = {}
            for h in range(H):
                if h < 8:
                    # reload xt chunk h for the deferred V columns
                    xtb = xtp.tile([128, KD, 512], BF, name="xt_v1", tag="xt")
                    nc.sync.dma_start(xtb[:], xsrc[:, :, 512 * h:512 * (h + 1)])
                    vg1_chunks[h] = xtb
                if h > 0:
                    emit_u(h - 1, 0, 16)
                emit_S(h, 0, 2)
                if h > 0:
                    emit_u(h - 1, 16, 32)
                emit_S(h, 2, 4)
                if h > 0:
                    emit_chain(h - 1)
                    emit_out(h - 1)
            emit_u(H - 1, 0, 16)
            emit_u(H - 1, 16, 32)
            emit_chain(H - 1)
            emit_out(H - 1)

            p3_scope.__exit__(None, None, None)
            # ---------- P4: final projection ----------
            p4_scope = nc.named_scope(f"p4_b{b}"); p4_scope.__enter__()
            osrc = ot_d[b].rearrange("(k p) n -> p k n", p=128)
            for c in range(NCH):
                oti = xtp.tile([128, KD, 512], BF, name="oti", tag="xt")
                nc.sync.dma_start(oti[:], osrc[:, :, 512 * c:512 * (c + 1)])
                for m in range(KD):
                    ps = psA.tile([128, 512], DT, name="f_ps", tag="A")
                    for k in range(KD):
                        nc.tensor.matmul(ps[:], wo_sb[:, k, 128 * m:128 * (m + 1)],
                                         oti[:, k, :],
                                         start=(k == 0), stop=(k == KD - 1))
                    st = fosg.tile([128, 512], DT, name="fin_st", tag="fin_st")
                    nc.scalar.activation(st[:], ps[:], AF.Identity,
                                         bias=bias_sb["bo"][:, m:m + 1])
                    nc.sync.dma_start(
                        out_d[b, 128 * m:128 * (m + 1), 512 * c:512 * (c + 1)],
                        st[:])
            p4_scope.__exit__(None, None, None)

    nc.compile()
    return nc


_nc_cache = {}


def _get_program(nb=NB, with_bv=False):
    key = (nb, with_bv)
    if key not in _nc_cache:
        _nc_cache[key] = build_program(nb, with_bv)
    return _nc_cache[key]


def make_in_maps(hidden_states, text_states, Wq, bq, Wk, bk, Wv, bv, Wo, bo):
    """Host-side staging: transpose to feature-major, shard batches."""
    import ml_dtypes
    f32 = np.float32
    bf16 = ml_dtypes.bfloat16
    hs = np.asarray(hidden_states, f32)
    ts = np.asarray(text_states, f32)
    xt_all = np.ascontiguousarray(hs.transpose(0, 2, 1)).astype(bf16)  # (B,D,LV)
    # Faithful to the reference's torch-style .view: text_states (LT, B, D)
    # reinterpreted in raw memory order as (B, LT, D), then feature-major.
    tt_all = np.ascontiguousarray(
        ts.reshape(B, LT, D).transpose(0, 2, 1)).astype(bf16)
    with_bv = bool(np.any(np.asarray(bv)))
    shared = {
        "wq": np.asarray(Wq, f32).astype(bf16),
        "wk": np.asarray(Wk, f32).astype(bf16),
        "wv": np.asarray(Wv, f32).astype(bf16),
        "wo": np.asarray(Wo, f32).astype(bf16),
        "bqs8": np.ascontiguousarray(np.asarray(bq, f32) * (SCALE * QK8)),
        "bk8": np.ascontiguousarray(np.asarray(bk, f32) * QK8),
        "bo": np.ascontiguousarray(np.asarray(bo, f32)),
    }
    if with_bv:
        shared["bv"] = np.ascontiguousarray(np.asarray(bv, f32))
    in_maps = []
    for c in range(N_CORES):
        sl = slice(c * NB, (c + 1) * NB)
        in_maps.append({
            "xt": np.ascontiguousarray(xt_all[sl]),
            "tt": np.ascontiguousarray(tt_all[sl]),
            **shared,
        })
    return in_maps, with_bv


def kernel(hidden_states, text_states, Wq, bq, Wk, bk, Wv, bv, Wo, bo):
    in_maps, with_bv = make_in_maps(hidden_states, text_states, Wq, bq,
                                    Wk, bk, Wv, bv, Wo, bo)
    nc = _get_program(with_bv=with_bv)
    res = bass_utils.run_bass_kernel_spmd(nc, in_maps, list(range(N_CORES)))
    out = np.empty((B, LV, D), np.float32)
    for c in range(N_CORES):
        o = res.results[c]["out"]                                  # (NB, D, LV)
        for j in range(NB):
            out[c * NB + j] = o[j].T
    return out
